# revision 13
# baseline (speedup 1.0000x reference)
"""KeypointFlowLoss Trainium2 kernel.

The loss only reads each flow at the K keypoint pixels the reference
scatters into the ground-truth image (everywhere else gt == 0, mask == 0),
so instead of streaming 5 x [16,2,512,512] f32 from HBM we gather exactly
the needed pixels with one indirect DMA per core and reduce on-chip.

Sharding: data-parallel over batch — core c owns batches [2c, 2c+2).
Host-side marshalling re-lays the five flows out as one [B,H,W,2,5] tensor
(per-core slice is a contiguous view) and precomputes, per core, a packed
[20,34] i32 aux block: rows 0-9 the gather index table (element index of
each keypoint's 10 flow values, transposed layout), rows 10-19 the bitcast
f32 keypoint displacements. Masked-out keypoints get out-of-bounds indices
(silently dropped by the gather, leaving memset zeros) and zero disp, so
they contribute exactly 0 to every sum with no mask multiply.

Device critical path per core (everything else overlaps):
  aux DMA -> indirect gather g[10,34] -> d=g-disp, d^2, pair-sum [5,34]
  -> ACT sqrt with free-axis accumulate -> [5,1] partial sums -> out DMA.
The host adds the 8 cores' partials, divides by the host-computed mask
count, and applies the gamma weighting.
"""

import numpy as np

import concourse.bacc as bacc
import concourse.bass as bass
import concourse.mybir as mybir
import concourse.tile as tile
from concourse.bass import IndirectOffsetOnAxis
from concourse.bass_utils import run_bass_kernel_spmd

B, CH, H, W = 16, 2, 512, 512
K = 17
NF = 5
NCORES = 8
BL = B // NCORES          # batches per core
NP = BL * K               # keypoints per core
NV = NF * CH              # flow values per keypoint
TOT = BL * H * W * NV     # per-core flow elements
GAMMA = 0.8
LOSS_WEIGHT = 1.0

F32 = mybir.dt.float32
I32 = mybir.dt.int32

_PROGRAM = None
_RUN_KWARGS = {}      # test harness can set {"trace": True} to profile
_LAST_RESULTS = None


def _build_program():
    nc = bacc.Bacc(None, target_bir_lowering=False)

    fs = nc.dram_tensor("fs", [TOT], F32, kind="ExternalInput")
    aux = nc.dram_tensor("aux", [NF, 3 * NP], I32, kind="ExternalInput")
    out = nc.dram_tensor("out", [NF, 1], F32, kind="ExternalOutput")

    with tile.TileContext(nc) as tc:
        with tc.tile_pool(name="sbuf", bufs=1) as sb:
            # cols 0-33: gather index table; cols 34-101: bitcast f32 disp
            at = sb.tile([NF, 3 * NP], I32)
            nc.sync.dma_start(out=at[:], in_=aux[:])
            disp = at[:, NP:3 * NP].bitcast(F32)

            # gather: each index fetches the contiguous (x,y) f32 pair of one
            # flow at one keypoint; OOB indices (masked keypoints) are dropped
            # and leave the memset zeros in place. g[f, 2i+c] = value.
            g = sb.tile([NF, 2 * NP], F32)
            nc.vector.memset(g[:], 0.0)
            # pair-granular view: index i reads elements [2i, 2i+1]
            flat = bass.AP(fs, 0, [[2, TOT // 2], [1, 2]])
            nc.gpsimd.indirect_dma_start(
                out=g[:],
                out_offset=None,
                in_=flat,
                in_offset=IndirectOffsetOnAxis(ap=at[:, 0:NP], axis=0),
                bounds_check=TOT // 2 - 1,
                oob_is_err=False,
            )

            # col 2i: x-diff of keypoint i; col 2i+1: y-diff
            d = sb.tile([NF, 2 * NP], F32)
            nc.vector.tensor_tensor(out=d[:], in0=g[:], in1=disp,
                                    op=mybir.AluOpType.subtract)
            nc.vector.tensor_tensor(out=d[:], in0=d[:], in1=d[:],
                                    op=mybir.AluOpType.mult)
            s = sb.tile([NF, NP], F32)
            nc.vector.tensor_tensor(out=s[:], in0=d[:, 0:2 * NP:2],
                                    in1=d[:, 1:2 * NP:2],
                                    op=mybir.AluOpType.add)

            # epe = sqrt(s); accum_out gives the per-flow keypoint sum
            epe = sb.tile([NF, NP], F32)
            res = sb.tile([NF, 1], F32)
            nc.scalar.activation(out=epe[:], in_=s[:],
                                 func=mybir.ActivationFunctionType.Sqrt,
                                 accum_out=res[:])
            nc.sync.dma_start(out=out[:], in_=res[:])

    nc.finalize()
    return nc


def _get_program():
    global _PROGRAM
    if _PROGRAM is None:
        _PROGRAM = _build_program()
    return _PROGRAM


def _shard_inputs(inputs):
    """Host-side marshalling: returns (in_maps for the 8 cores, mask count)."""
    flows = [np.asarray(inputs[f"flow{i}"], dtype=np.float32) for i in range(NF)]
    kps = np.asarray(inputs["kps"], dtype=np.int64)

    # T[b,y,x,f,c] = flow_f[b,c,y,x]; per-core slice stays a contiguous view.
    t = np.ascontiguousarray(
        np.stack(flows, axis=0).transpose(1, 3, 4, 0, 2)
    ).reshape(B, H * W * NV)

    kps0, kps1 = kps[:, 0], kps[:, 1]        # [B, K, 2] (x, y)
    x0, y0 = kps0[..., 0], kps0[..., 1]
    x1, y1 = kps1[..., 0], kps1[..., 1]
    valid = (
        (kps0 >= 0).all(-1) & (kps1 >= 0).all(-1)
        & (x0 < W) & (y0 < H) & (x1 < W) & (y1 < H)
    )
    disp = (kps1 - kps0).astype(np.float32)  # [B, K, 2]
    mask = valid & (kps1 != kps0).any(-1)    # [B, K]
    disp[~mask] = 0.0

    # pair index of (b, y0, x0)'s first flow pair; OOB when masked out
    idx = np.where(mask, (y0 * W + x0) * NF, TOT).astype(np.int64)   # [B, K]

    in_maps = []
    for c in range(NCORES):
        sl = slice(c * BL, (c + 1) * BL)
        loc = idx[sl] + (np.arange(BL) * (H * W * NF))[:, None]   # [BL, K]
        aux = np.empty((NF, 3 * NP), dtype=np.int32)
        # cols 0-33: pair-index of flow f's (x,y) pair of keypoint i at [f, i]
        f_off = np.arange(NF, dtype=np.int64)[:, None]            # [NF, 1]
        aux[:, 0:NP] = (loc.reshape(1, NP) + f_off).astype(np.int32)
        # cols 34-101: bitcast f32 disp, (dx_i, dy_i) interleaved, same per row
        dv = disp[sl].reshape(1, 2 * NP).view(np.int32)
        aux[:, NP:3 * NP] = dv
        in_maps.append({"fs": t[sl].reshape(TOT), "aux": aux})
    return in_maps, float(mask.sum())


def kernel(**inputs):
    in_maps, cnt = _shard_inputs(inputs)
    nc = _get_program()

    results = run_bass_kernel_spmd(nc, in_maps, core_ids=list(range(NCORES)),
                                   **_RUN_KWARGS)
    globals()["_LAST_RESULTS"] = results

    sums = np.zeros(NF, dtype=np.float32)
    for r in results.results:
        sums += r["out"].reshape(-1).astype(np.float32)

    weights = (np.float32(GAMMA) ** np.arange(NF - 1, -1, -1, dtype=np.float32))
    means = sums / np.float32(cnt)
    loss = np.float32(np.sum(weights * means, dtype=np.float32) * np.float32(LOSS_WEIGHT))
    return np.asarray(loss, dtype=np.float32)


# revision 24
# speedup vs baseline: 1.0372x; 1.0372x over previous
"""KeypointFlowLoss Trainium2 kernel.

The loss only reads each flow at the K keypoint pixels the reference
scatters into the ground-truth image (everywhere else gt == 0, mask == 0),
so instead of streaming 5 x [16,2,512,512] f32 from HBM we gather exactly
the needed pixels with one indirect DMA per core and reduce on-chip.

Sharding: data-parallel over batch — core c owns batches [2c, 2c+2).
Host-side marshalling re-lays the five flows out as one [B,H,W,2,5] tensor
(per-core slice is a contiguous view) and precomputes, per core, a packed
[20,34] i32 aux block: rows 0-9 the gather index table (element index of
each keypoint's 10 flow values, transposed layout), rows 10-19 the bitcast
f32 keypoint displacements. Masked-out keypoints get out-of-bounds indices
(silently dropped by the gather, leaving memset zeros) and zero disp, so
they contribute exactly 0 to every sum with no mask multiply.

Device critical path per core (everything else overlaps):
  aux DMA -> indirect gather g[10,34] -> d=g-disp, d^2, pair-sum [5,34]
  -> ACT sqrt with free-axis accumulate -> [5,1] partial sums -> out DMA.
The host adds the 8 cores' partials, divides by the host-computed mask
count, and applies the gamma weighting.
"""

import numpy as np

import concourse.bacc as bacc
import concourse.bass as bass
import concourse.mybir as mybir
import concourse.tile as tile
from concourse.bass import IndirectOffsetOnAxis
from concourse.bass_utils import run_bass_kernel_spmd

B, CH, H, W = 16, 2, 512, 512
K = 17
NF = 5
NCORES = 8
BL = B // NCORES          # batches per core
NP = BL * K               # keypoints per core
NV = NF * CH              # flow values per keypoint
TOT = BL * H * W * NV     # per-core flow elements
GAMMA = 0.8
LOSS_WEIGHT = 1.0

F32 = mybir.dt.float32
I32 = mybir.dt.int32

_PROGRAM = None
_RUN_KWARGS = {}      # test harness can set {"trace": True} to profile
_LAST_RESULTS = None


def _build_program():
    """Raw-bass program (no TileContext): hand-placed semaphores, no entry or
    exit all-engine barriers. Every cross-engine edge is a producer .then_inc
    paired with a consumer wait that also decrements, so all semaphores read
    zero again at program end and the NEFF is safely re-executable."""
    nc = bacc.Bacc(None, target_bir_lowering=False)

    fs = nc.dram_tensor("fs", [TOT], F32, kind="ExternalInput")
    aux = nc.dram_tensor("aux", [NF, 3 * NP], I32, kind="ExternalInput")
    out = nc.dram_tensor("out", [NF, 1], F32, kind="ExternalOutput")

    s_at = nc.alloc_semaphore("s_at")   # aux table landed in SBUF
    s_ms = nc.alloc_semaphore("s_ms")   # g memset done
    s_g = nc.alloc_semaphore("s_g")     # gather landed
    s_s = nc.alloc_semaphore("s_s")     # s ready for sqrt
    s_e = nc.alloc_semaphore("s_e")     # per-flow sums ready
    s_o = nc.alloc_semaphore("s_o")     # out DMA landed
    s_v = nc.alloc_semaphore("s_v")     # DVE same-engine RAW chain
    sems = [s_at, s_ms, s_g, s_s, s_e, s_o, s_v]

    with (
        nc.sbuf_tensor([NF, 3 * NP], I32) as at,
        nc.sbuf_tensor([NF, 2 * NP], F32) as g,
        nc.sbuf_tensor([NF, 2 * NP], F32) as d,
        nc.sbuf_tensor([NF, NP], F32) as s,
        nc.sbuf_tensor([NF, NP], F32) as epe,
        nc.sbuf_tensor([NF, 1], F32) as res,
    ):
        # cols 0-33: gather index table; cols 34-101: bitcast f32 disp
        nc.sync.dma_start(out=at[:], in_=aux[:]).then_inc(s_at, 16)
        disp = at[:, NP:3 * NP].bitcast(F32)

        nc.vector.memset(g[:], 0.0)
        nc.vector.engine_nop().then_inc(s_ms, 1)

        # gather: each index fetches the contiguous (x,y) f32 pair of one
        # flow at one keypoint; OOB indices (masked keypoints) are dropped
        # and leave the memset zeros in place. g[f, 2i+c] = value.
        nc.gpsimd.wait_ge(s_at, 16)
        nc.gpsimd.wait_ge(s_ms, 1)
        flat = bass.AP(fs, 0, [[2, TOT // 2], [1, 2]])
        nc.gpsimd.indirect_dma_start(
            out=g[:],
            out_offset=None,
            in_=flat,
            in_offset=IndirectOffsetOnAxis(ap=at[:, 0:NP], axis=0),
            bounds_check=TOT // 2 - 1,
            oob_is_err=False,
        ).then_inc(s_g, 16)

        # col 2i: x-diff of keypoint i; col 2i+1: y-diff
        # Semaphore decrements ride on instructions that retire strictly
        # after the protected data's last reader, restoring every semaphore
        # to zero by program end (NEFF re-execution safety).
        nc.vector.wait_ge(s_g, 16)
        nc.vector.tensor_tensor(out=d[:], in0=g[:], in1=disp,
                                op=mybir.AluOpType.subtract).then_inc(s_v, 1)
        nc.vector.wait_ge(s_v, 1)
        nc.vector.tensor_tensor(out=d[:], in0=d[:], in1=d[:],
                                op=mybir.AluOpType.mult).then_inc(s_v, 1)
        nc.vector.wait_ge(s_v, 2)
        nc.vector.tensor_tensor(out=s[:], in0=d[:, 0:2 * NP:2],
                                in1=d[:, 1:2 * NP:2],
                                op=mybir.AluOpType.add).then_inc(s_s, 1)

        # epe = sqrt(s); accum_out gives the per-flow keypoint sum
        nc.scalar.wait_ge(s_s, 1)
        nc.scalar.activation(out=epe[:], in_=s[:],
                             func=mybir.ActivationFunctionType.Sqrt,
                             accum_out=res[:]).then_inc(s_e, 1)

        nc.sync.wait_ge(s_e, 1)
        nc.sync.dma_start(out=out[:], in_=res[:]).then_inc(s_o, 16)
        # one barrier so every engine's updates retire, then a single
        # range-clear resets the semaphores for NEFF re-execution
        nc.sync.wait_ge(s_o, 16)
        nc.all_engine_barrier()
        lo = min(sm.num for sm in sems)
        hi = max(sm.num for sm in sems)
        nc.sync.sem_clear(range(lo, hi + 1))

    nc.finalize()
    return nc


def _get_program():
    global _PROGRAM
    if _PROGRAM is None:
        _PROGRAM = _build_program()
    return _PROGRAM


def _shard_inputs(inputs):
    """Host-side marshalling: returns (in_maps for the 8 cores, mask count)."""
    flows = [np.asarray(inputs[f"flow{i}"], dtype=np.float32) for i in range(NF)]
    kps = np.asarray(inputs["kps"], dtype=np.int64)

    # T[b,y,x,f,c] = flow_f[b,c,y,x]; per-core slice stays a contiguous view.
    t = np.ascontiguousarray(
        np.stack(flows, axis=0).transpose(1, 3, 4, 0, 2)
    ).reshape(B, H * W * NV)

    kps0, kps1 = kps[:, 0], kps[:, 1]        # [B, K, 2] (x, y)
    x0, y0 = kps0[..., 0], kps0[..., 1]
    x1, y1 = kps1[..., 0], kps1[..., 1]
    valid = (
        (kps0 >= 0).all(-1) & (kps1 >= 0).all(-1)
        & (x0 < W) & (y0 < H) & (x1 < W) & (y1 < H)
    )
    disp = (kps1 - kps0).astype(np.float32)  # [B, K, 2]
    mask = valid & (kps1 != kps0).any(-1)    # [B, K]
    disp[~mask] = 0.0

    # pair index of (b, y0, x0)'s first flow pair; OOB when masked out
    idx = np.where(mask, (y0 * W + x0) * NF, TOT).astype(np.int64)   # [B, K]

    in_maps = []
    for c in range(NCORES):
        sl = slice(c * BL, (c + 1) * BL)
        loc = idx[sl] + (np.arange(BL) * (H * W * NF))[:, None]   # [BL, K]
        aux = np.empty((NF, 3 * NP), dtype=np.int32)
        # cols 0-33: pair-index of flow f's (x,y) pair of keypoint i at [f, i]
        f_off = np.arange(NF, dtype=np.int64)[:, None]            # [NF, 1]
        aux[:, 0:NP] = (loc.reshape(1, NP) + f_off).astype(np.int32)
        # cols 34-101: bitcast f32 disp, (dx_i, dy_i) interleaved, same per row
        dv = disp[sl].reshape(1, 2 * NP).view(np.int32)
        aux[:, NP:3 * NP] = dv
        in_maps.append({"fs": t[sl].reshape(TOT), "aux": aux})
    return in_maps, float(mask.sum())


def kernel(**inputs):
    in_maps, cnt = _shard_inputs(inputs)
    nc = _get_program()

    results = run_bass_kernel_spmd(nc, in_maps, core_ids=list(range(NCORES)),
                                   **_RUN_KWARGS)
    globals()["_LAST_RESULTS"] = results

    sums = np.zeros(NF, dtype=np.float32)
    for r in results.results:
        sums += r["out"].reshape(-1).astype(np.float32)

    weights = (np.float32(GAMMA) ** np.arange(NF - 1, -1, -1, dtype=np.float32))
    means = sums / np.float32(cnt)
    loss = np.float32(np.sum(weights * means, dtype=np.float32) * np.float32(LOSS_WEIGHT))
    return np.asarray(loss, dtype=np.float32)


# revision 30
# speedup vs baseline: 1.0495x; 1.0119x over previous
"""KeypointFlowLoss Trainium2 kernel.

The loss only reads each flow at the K keypoint pixels the reference
scatters into the ground-truth image (everywhere else gt == 0, mask == 0),
so instead of streaming 5 x [16,2,512,512] f32 from HBM we gather exactly
the needed pixels with one indirect DMA per core and reduce on-chip.

Sharding: data-parallel over batch — core c owns batches [2c, 2c+2).
Host-side marshalling re-lays the five flows out as one [B,H,W,2,5] tensor
(per-core slice is a contiguous view) and precomputes, per core, a packed
[20,34] i32 aux block: rows 0-9 the gather index table (element index of
each keypoint's 10 flow values, transposed layout), rows 10-19 the bitcast
f32 keypoint displacements. Masked-out keypoints get out-of-bounds indices
(silently dropped by the gather, leaving memset zeros) and zero disp, so
they contribute exactly 0 to every sum with no mask multiply.

Device critical path per core (everything else overlaps):
  aux DMA -> indirect gather g[10,34] -> d=g-disp, d^2, pair-sum [5,34]
  -> ACT sqrt with free-axis accumulate -> [5,1] partial sums -> out DMA.
The host adds the 8 cores' partials, divides by the host-computed mask
count, and applies the gamma weighting.
"""

import numpy as np

import concourse.bacc as bacc
import concourse.bass as bass
import concourse.mybir as mybir
import concourse.tile as tile
from concourse.bass import IndirectOffsetOnAxis
from concourse.bass_utils import run_bass_kernel_spmd

B, CH, H, W = 16, 2, 512, 512
K = 17
NF = 5
NCORES = 8
BL = B // NCORES          # batches per core
NP = BL * K               # keypoints per core
NV = NF * CH              # flow values per keypoint
TOT = BL * H * W * NV     # per-core flow elements
GAMMA = 0.8
LOSS_WEIGHT = 1.0

F32 = mybir.dt.float32
I32 = mybir.dt.int32

_PROGRAM = None
_RUN_KWARGS = {}      # test harness can set {"trace": True} to profile
_LAST_RESULTS = None


def _build_program():
    """Raw-bass program (no TileContext): hand-placed semaphores, no entry or
    exit all-engine barriers. Every cross-engine edge is a producer .then_inc
    paired with a consumer wait that also decrements, so all semaphores read
    zero again at program end and the NEFF is safely re-executable."""
    nc = bacc.Bacc(None, target_bir_lowering=False)

    fs = nc.dram_tensor("fs", [TOT], F32, kind="ExternalInput")
    aux = nc.dram_tensor("aux", [NF, 3 * NP], I32, kind="ExternalInput")
    out = nc.dram_tensor("out", [NF, 1], F32, kind="ExternalOutput")

    s_rdy = nc.alloc_semaphore("s_rdy")  # aux table (+16) and g memset (+1)
    s_g = nc.alloc_semaphore("s_g")     # gather landed
    s_o = nc.alloc_semaphore("s_o")     # out DMA landed
    s_v = nc.alloc_semaphore("s_v")     # compute chain progress counter
    sems = [s_rdy, s_g, s_o, s_v]

    with (
        nc.sbuf_tensor([NF, 3 * NP], I32) as at,
        nc.sbuf_tensor([NF, 2 * NP], F32) as g,
        nc.sbuf_tensor([NF, 2 * NP], F32) as d,
        nc.sbuf_tensor([NF, NP], F32) as s,
        nc.sbuf_tensor([NF, NP], F32) as epe,
        nc.sbuf_tensor([NF, 1], F32) as res,
    ):
        # cols 0-33: gather index table; cols 34-101: bitcast f32 disp
        nc.sync.dma_start(out=at[:], in_=aux[:]).then_inc(s_rdy, 16)
        disp = at[:, NP:3 * NP].bitcast(F32)

        nc.vector.memset(g[:], 0.0).then_inc(s_rdy, 1)

        # gather: each index fetches the contiguous (x,y) f32 pair of one
        # flow at one keypoint; OOB indices (masked keypoints) are dropped
        # and leave the memset zeros in place. g[f, 2i+c] = value.
        flat = bass.AP(fs, 0, [[2, TOT // 2], [1, 2]])
        nc.gpsimd.indirect_dma_start(
            out=g[:],
            out_offset=None,
            in_=flat,
            in_offset=IndirectOffsetOnAxis(ap=at[:, 0:NP], axis=0),
            bounds_check=TOT // 2 - 1,
            oob_is_err=False,
        ).wait_op(s_rdy, 17, "sem-ge").then_inc(s_g, 16)

        # col 2i: x-diff of keypoint i; col 2i+1: y-diff
        # Semaphore decrements ride on instructions that retire strictly
        # after the protected data's last reader, restoring every semaphore
        # to zero by program end (NEFF re-execution safety).
        nc.vector.tensor_tensor(out=d[:], in0=g[:], in1=disp,
                                op=mybir.AluOpType.subtract) \
            .wait_op(s_g, 16, "sem-ge").then_inc(s_v, 1)
        nc.vector.tensor_tensor(out=d[:], in0=d[:], in1=d[:],
                                op=mybir.AluOpType.mult) \
            .wait_op(s_v, 1, "sem-ge").then_inc(s_v, 1)
        nc.vector.tensor_tensor(out=s[:], in0=d[:, 0:2 * NP:2],
                                in1=d[:, 1:2 * NP:2],
                                op=mybir.AluOpType.add) \
            .wait_op(s_v, 2, "sem-ge").then_inc(s_v, 1)

        # epe = sqrt(s); accum_out gives the per-flow keypoint sum
        nc.scalar.activation(out=epe[:], in_=s[:],
                             func=mybir.ActivationFunctionType.Sqrt,
                             accum_out=res[:]) \
            .wait_op(s_v, 3, "sem-ge").then_inc(s_v, 1)

        nc.sync.dma_start(out=out[:], in_=res[:]) \
            .wait_op(s_v, 4, "sem-ge").then_inc(s_o, 16)
        # one barrier so every engine's updates retire, then a single
        # range-clear resets the semaphores for NEFF re-execution
        nc.sync.wait_ge(s_o, 16)
        nc.all_engine_barrier()
        lo = min(sm.num for sm in sems)
        hi = max(sm.num for sm in sems)
        nc.sync.sem_clear(range(lo, hi + 1))

    nc.finalize()
    return nc


def _get_program():
    global _PROGRAM
    if _PROGRAM is None:
        _PROGRAM = _build_program()
    return _PROGRAM


def _shard_inputs(inputs):
    """Host-side marshalling: returns (in_maps for the 8 cores, mask count)."""
    flows = [np.asarray(inputs[f"flow{i}"], dtype=np.float32) for i in range(NF)]
    kps = np.asarray(inputs["kps"], dtype=np.int64)

    # T[b,y,x,f,c] = flow_f[b,c,y,x]; per-core slice stays a contiguous view.
    t = np.ascontiguousarray(
        np.stack(flows, axis=0).transpose(1, 3, 4, 0, 2)
    ).reshape(B, H * W * NV)

    kps0, kps1 = kps[:, 0], kps[:, 1]        # [B, K, 2] (x, y)
    x0, y0 = kps0[..., 0], kps0[..., 1]
    x1, y1 = kps1[..., 0], kps1[..., 1]
    valid = (
        (kps0 >= 0).all(-1) & (kps1 >= 0).all(-1)
        & (x0 < W) & (y0 < H) & (x1 < W) & (y1 < H)
    )
    disp = (kps1 - kps0).astype(np.float32)  # [B, K, 2]
    mask = valid & (kps1 != kps0).any(-1)    # [B, K]
    disp[~mask] = 0.0

    # pair index of (b, y0, x0)'s first flow pair; OOB when masked out
    idx = np.where(mask, (y0 * W + x0) * NF, TOT).astype(np.int64)   # [B, K]

    in_maps = []
    for c in range(NCORES):
        sl = slice(c * BL, (c + 1) * BL)
        loc = idx[sl] + (np.arange(BL) * (H * W * NF))[:, None]   # [BL, K]
        aux = np.empty((NF, 3 * NP), dtype=np.int32)
        # cols 0-33: pair-index of flow f's (x,y) pair of keypoint i at [f, i]
        f_off = np.arange(NF, dtype=np.int64)[:, None]            # [NF, 1]
        aux[:, 0:NP] = (loc.reshape(1, NP) + f_off).astype(np.int32)
        # cols 34-101: bitcast f32 disp, (dx_i, dy_i) interleaved, same per row
        dv = disp[sl].reshape(1, 2 * NP).view(np.int32)
        aux[:, NP:3 * NP] = dv
        in_maps.append({"fs": t[sl].reshape(TOT), "aux": aux})
    return in_maps, float(mask.sum())


def kernel(**inputs):
    in_maps, cnt = _shard_inputs(inputs)
    nc = _get_program()

    results = run_bass_kernel_spmd(nc, in_maps, core_ids=list(range(NCORES)),
                                   **_RUN_KWARGS)
    globals()["_LAST_RESULTS"] = results

    sums = np.zeros(NF, dtype=np.float32)
    for r in results.results:
        sums += r["out"].reshape(-1).astype(np.float32)

    weights = (np.float32(GAMMA) ** np.arange(NF - 1, -1, -1, dtype=np.float32))
    means = sums / np.float32(cnt)
    loss = np.float32(np.sum(weights * means, dtype=np.float32) * np.float32(LOSS_WEIGHT))
    return np.asarray(loss, dtype=np.float32)


# revision 35
# speedup vs baseline: 1.1042x; 1.0521x over previous
"""KeypointFlowLoss Trainium2 kernel.

The loss only reads each flow at the K keypoint pixels the reference
scatters into the ground-truth image (everywhere else gt == 0, mask == 0),
so instead of streaming 5 x [16,2,512,512] f32 from HBM we gather exactly
the needed pixels with one indirect DMA per core and reduce on-chip.

Sharding: data-parallel over batch — core c owns batches [2c, 2c+2).
Host-side marshalling re-lays the five flows out as one [B,H,W,2,5] tensor
(per-core slice is a contiguous view) and precomputes, per core, a packed
[20,34] i32 aux block: rows 0-9 the gather index table (element index of
each keypoint's 10 flow values, transposed layout), rows 10-19 the bitcast
f32 keypoint displacements. Masked-out keypoints get out-of-bounds indices
(silently dropped by the gather, leaving memset zeros) and zero disp, so
they contribute exactly 0 to every sum with no mask multiply.

Device critical path per core (everything else overlaps):
  aux DMA -> indirect gather g[10,34] -> d=g-disp, d^2, pair-sum [5,34]
  -> ACT sqrt with free-axis accumulate -> [5,1] partial sums -> out DMA.
The host adds the 8 cores' partials, divides by the host-computed mask
count, and applies the gamma weighting.
"""

import numpy as np

import concourse.bacc as bacc
import concourse.bass as bass
import concourse.mybir as mybir
import concourse.tile as tile
from concourse.bass import IndirectOffsetOnAxis
from concourse.bass_utils import run_bass_kernel_spmd

B, CH, H, W = 16, 2, 512, 512
K = 17
NF = 5
NCORES = 8
BL = B // NCORES          # batches per core
NP = BL * K               # keypoints per core
NV = NF * CH              # flow values per keypoint
TOT = BL * H * W * NV     # per-core flow elements
GAMMA = 0.8
LOSS_WEIGHT = 1.0

F32 = mybir.dt.float32
I32 = mybir.dt.int32

_PROGRAM = None
_RUN_KWARGS = {}      # test harness can set {"trace": True} to profile
_LAST_RESULTS = None


def _build_program():
    """Raw-bass program (no TileContext): hand-placed semaphores, no entry or
    exit all-engine barriers. Every cross-engine edge is a producer .then_inc
    paired with a consumer wait that also decrements, so all semaphores read
    zero again at program end and the NEFF is safely re-executable."""
    nc = bacc.Bacc(None, target_bir_lowering=False)

    # Strip the constructor's entry all-engine barrier: every cross-engine
    # edge below carries an explicit semaphore, and the const-AP pool the
    # barrier protects is not used (the activation bias is an own memset
    # tile synced through the s_v chain).
    entry = nc.main_func.blocks[0]
    for inst in [i for i in entry.instructions
                 if isinstance(i, (mybir.InstDrain, mybir.InstEventSemaphore))]:
        entry.instructions.remove(inst)

    fs = nc.dram_tensor("fs", [TOT], F32, kind="ExternalInput")
    aux = nc.dram_tensor("aux", [NF, 3 * NP], I32, kind="ExternalInput")
    out = nc.dram_tensor("out", [NF, 1], F32, kind="ExternalOutput")

    s_rdy = nc.alloc_semaphore("s_rdy")  # aux table (+16) and g memset (+1)
    s_g = nc.alloc_semaphore("s_g")     # gather landed
    s_o = nc.alloc_semaphore("s_o")     # out DMA landed
    s_v = nc.alloc_semaphore("s_v")     # compute chain progress counter
    sems = [s_rdy, s_g, s_o, s_v]

    with (
        nc.sbuf_tensor([NF, 3 * NP], I32) as at,
        nc.sbuf_tensor([NF, 2 * NP], F32) as g,
        nc.sbuf_tensor([NF, 2 * NP], F32) as d,
        nc.sbuf_tensor([NF, NP], F32) as s,
        nc.sbuf_tensor([NF, NP], F32) as epe,
        nc.sbuf_tensor([NF, 1], F32) as res,
        nc.sbuf_tensor([NF, 1], F32) as zb,
    ):
        # cols 0-33: gather index table; cols 34-101: bitcast f32 disp
        nc.sync.dma_start(out=at[:], in_=aux[:]).then_inc(s_rdy, 16)
        disp = at[:, NP:3 * NP].bitcast(F32)

        nc.vector.memset(zb[:], 0.0)   # activation bias zeros (synced via s_v)
        nc.vector.memset(g[:], 0.0).then_inc(s_rdy, 1)

        # gather: each index fetches the contiguous (x,y) f32 pair of one
        # flow at one keypoint; OOB indices (masked keypoints) are dropped
        # and leave the memset zeros in place. g[f, 2i+c] = value.
        flat = bass.AP(fs, 0, [[2, TOT // 2], [1, 2]])
        nc.gpsimd.indirect_dma_start(
            out=g[:],
            out_offset=None,
            in_=flat,
            in_offset=IndirectOffsetOnAxis(ap=at[:, 0:NP], axis=0),
            bounds_check=TOT // 2 - 1,
            oob_is_err=False,
        ).wait_op(s_rdy, 17, "sem-ge").then_inc(s_g, 16)

        # col 2i: x-diff of keypoint i; col 2i+1: y-diff
        # Semaphore decrements ride on instructions that retire strictly
        # after the protected data's last reader, restoring every semaphore
        # to zero by program end (NEFF re-execution safety).
        nc.vector.tensor_tensor(out=d[:], in0=g[:], in1=disp,
                                op=mybir.AluOpType.subtract) \
            .wait_op(s_g, 16, "sem-ge").then_inc(s_v, 1)
        nc.vector.tensor_tensor(out=d[:], in0=d[:], in1=d[:],
                                op=mybir.AluOpType.mult) \
            .wait_op(s_v, 1, "sem-ge").then_inc(s_v, 1)
        nc.vector.tensor_tensor(out=s[:], in0=d[:, 0:2 * NP:2],
                                in1=d[:, 1:2 * NP:2],
                                op=mybir.AluOpType.add) \
            .wait_op(s_v, 2, "sem-ge").then_inc(s_v, 1)

        # epe = sqrt(s); accum_out gives the per-flow keypoint sum
        nc.scalar.activation(out=epe[:], in_=s[:],
                             func=mybir.ActivationFunctionType.Sqrt,
                             bias=zb[:], accum_out=res[:]) \
            .wait_op(s_v, 3, "sem-ge").then_inc(s_v, 1)

        nc.sync.dma_start(out=out[:], in_=res[:]) \
            .wait_op(s_v, 4, "sem-ge").then_inc(s_o, 16)
        # one barrier so every engine's updates retire, then a single
        # range-clear resets the semaphores for NEFF re-execution
        nc.sync.wait_ge(s_o, 16)
        nc.all_engine_barrier(sem_only=True)
        lo = min(sm.num for sm in sems)
        hi = max(sm.num for sm in sems)
        nc.sync.sem_clear(range(lo, hi + 1))

    nc.finalize()
    return nc


def _get_program():
    global _PROGRAM
    if _PROGRAM is None:
        _PROGRAM = _build_program()
    return _PROGRAM


def _shard_inputs(inputs):
    """Host-side marshalling: returns (in_maps for the 8 cores, mask count)."""
    flows = [np.asarray(inputs[f"flow{i}"], dtype=np.float32) for i in range(NF)]
    kps = np.asarray(inputs["kps"], dtype=np.int64)

    # T[b,y,x,f,c] = flow_f[b,c,y,x]; per-core slice stays a contiguous view.
    t = np.ascontiguousarray(
        np.stack(flows, axis=0).transpose(1, 3, 4, 0, 2)
    ).reshape(B, H * W * NV)

    kps0, kps1 = kps[:, 0], kps[:, 1]        # [B, K, 2] (x, y)
    x0, y0 = kps0[..., 0], kps0[..., 1]
    x1, y1 = kps1[..., 0], kps1[..., 1]
    valid = (
        (kps0 >= 0).all(-1) & (kps1 >= 0).all(-1)
        & (x0 < W) & (y0 < H) & (x1 < W) & (y1 < H)
    )
    disp = (kps1 - kps0).astype(np.float32)  # [B, K, 2]
    mask = valid & (kps1 != kps0).any(-1)    # [B, K]
    disp[~mask] = 0.0

    # pair index of (b, y0, x0)'s first flow pair; OOB when masked out
    idx = np.where(mask, (y0 * W + x0) * NF, TOT).astype(np.int64)   # [B, K]

    in_maps = []
    for c in range(NCORES):
        sl = slice(c * BL, (c + 1) * BL)
        loc = idx[sl] + (np.arange(BL) * (H * W * NF))[:, None]   # [BL, K]
        aux = np.empty((NF, 3 * NP), dtype=np.int32)
        # cols 0-33: pair-index of flow f's (x,y) pair of keypoint i at [f, i]
        f_off = np.arange(NF, dtype=np.int64)[:, None]            # [NF, 1]
        aux[:, 0:NP] = (loc.reshape(1, NP) + f_off).astype(np.int32)
        # cols 34-101: bitcast f32 disp, (dx_i, dy_i) interleaved, same per row
        dv = disp[sl].reshape(1, 2 * NP).view(np.int32)
        aux[:, NP:3 * NP] = dv
        in_maps.append({"fs": t[sl].reshape(TOT), "aux": aux})
    return in_maps, float(mask.sum())


def kernel(**inputs):
    in_maps, cnt = _shard_inputs(inputs)
    nc = _get_program()

    results = run_bass_kernel_spmd(nc, in_maps, core_ids=list(range(NCORES)),
                                   **_RUN_KWARGS)
    globals()["_LAST_RESULTS"] = results

    sums = np.zeros(NF, dtype=np.float32)
    for r in results.results:
        sums += r["out"].reshape(-1).astype(np.float32)

    weights = (np.float32(GAMMA) ** np.arange(NF - 1, -1, -1, dtype=np.float32))
    means = sums / np.float32(cnt)
    loss = np.float32(np.sum(weights * means, dtype=np.float32) * np.float32(LOSS_WEIGHT))
    return np.asarray(loss, dtype=np.float32)
